# revision 11
# baseline (speedup 1.0000x reference)
"""Deformable-DETR encoder layer, fused + pipelined on 8 trn2 NeuronCores.

All compute runs on-device (projections, softmax, bilinear gather-blend,
output projection, LayerNorms, FFN). The axon tunnel is half-duplex at
~55 MB/s, so total wire bytes dominate; the kernel minimizes them:

  up:   src bf16 (13.6MB) | per-group [pos int4-packed | ref uint16] as
        one uint8 buffer (3.9MB) | weights bf16 sharded (1.5MB,
        all-gathered on device) | biases f32
  down: pre-affine LN2 output as int8 (6.8MB, fixed 3.9-sigma scale);
        the final `z*ln2_w + ln2_b` affine is applied on host in f32.

Pipeline: a tiny program all-gathers the sharded weight upload while src
streams; stage 0 computes value = src@Wv, all-gathers it within each
batch group, and processes the first token group; stages 1..G-1 reuse
the gathered value, overlapping execution with the pos uploads and the
int8 output downloads.

Sharding: mesh (b=2, c=4) — data-parallel over batch, token-parallel
within a batch group; the flattened multi-scale value memory is
replicated within the group by the on-device all-gather.
"""
import functools

import jax
import jax.numpy as jnp
import numpy as np
from jax.experimental.shard_map import shard_map
from jax.sharding import Mesh, NamedSharding, PartitionSpec as P

SHAPES = ((100, 100), (50, 50), (25, 25), (13, 13))
B, D, NH, NL, NP, DFF = 2, 256, 8, 4, 4, 1024
DH = D // NH
S = sum(h * w for h, w in SHAPES)  # 13294
NCHUNK = 4
G = 4                              # pipeline groups per core
SC = ((S + NCHUNK * G - 1) // (NCHUNK * G)) * G  # 3324 per-core tokens
SPAD = SC * NCHUNK                 # 13296 padded per-batch tokens
SG = SC // G                       # 831 tokens per core per group

Z_CLIP = 3.9                       # int8 clip for the unit-variance LN output
Z_SCALE = 127.0 / Z_CLIP

# per-group aux buffer: [pos int4 packed D//2 | ref hi | ref lo] bytes per token
_POS_B = D // 2
_REF_B = NL * 2
_AUX_B = _POS_B + 2 * _REF_B

_LVL_START = np.cumsum([0] + [h * w for h, w in SHAPES])

_WSEG = {}
_off = 0
for _name, _shape in (("w_value", (D, D)), ("w_off", (D, D)), ("w_attn", (D, NH * NL * NP)),
                      ("w_out", (D, D)), ("w_ff1", (D, DFF)), ("w_ff2", (DFF, D))):
    _WSEG[_name] = (_off, _shape)
    _off += _shape[0] * _shape[1]
_WTOT = _off
assert _WTOT % 8 == 0

_BSEG = {}
_off = 0
for _name, _n in (("b_value", D), ("b_off", D), ("b_attn", NH * NL * NP), ("b_out", D),
                  ("b_ff1", DFF), ("b_ff2", D), ("ln1_w", D), ("ln1_b", D),
                  ("s_pos", 1), ("s_src", 1)):
    _BSEG[_name] = (_off, _n)
    _off += _n
_BTOT = _off

_BF = jnp.bfloat16
_F32 = jnp.float32


@functools.lru_cache(maxsize=1)
def _mesh():
    devs = np.array(jax.devices()[:8]).reshape(2, 4)
    return Mesh(devs, ("b", "c"))


def _wseg(wall, name):
    o, shp = _WSEG[name]
    return wall[o:o + shp[0] * shp[1]].reshape(shp)


def _bseg(small, name):
    o, n = _BSEG[name]
    return small[o:o + n]


@functools.lru_cache(maxsize=1)
def _wgather_prog():
    mesh = _mesh()

    def body(wmat):
        wall_c = jax.lax.all_gather(wmat[0, 0], "c", axis=0)
        wall = jax.lax.all_gather(wall_c, "b", axis=0).reshape(-1)
        return wall[None]

    fn = shard_map(body, mesh=mesh, in_specs=(P("b", "c"),),
                   out_specs=P(None), check_rep=False)
    return jax.jit(fn)


def _group_body(vg, wall, small, s_q, aux):
    """Per-group compute: q/off/attn, bilinear gather-blend, out-proj,
    LN1, FFN, pre-affine LN2 -> int8. All inputs are per-core blocks."""
    s_pos = _bseg(small, "s_pos")[0]
    s_src = _bseg(small, "s_src")[0]
    s_ = s_q.astype(_F32) * s_src
    u = aux[:, :_POS_B]
    hi = (jnp.right_shift(u, 4)).astype(_F32) - 8.0
    lo = (jnp.bitwise_and(u, 15)).astype(_F32) - 8.0
    pq = jnp.stack([hi, lo], -1).reshape(SG, D) * s_pos
    rhi = aux[:, _POS_B:_POS_B + _REF_B].astype(_F32)
    rlo = aux[:, _POS_B + _REF_B:].astype(_F32)
    r = ((rhi * 256.0 + rlo) * (1.0 / 65535.0)).reshape(SG, NL, 2)

    q = (s_ + pq).astype(_BF)
    mm = lambda a, w: jnp.dot(a, w, preferred_element_type=_F32)
    off = (mm(q, _wseg(wall, "w_off")) + _bseg(small, "b_off")).reshape(SG, NH, NL, NP, 2)
    logits = (mm(q, _wseg(wall, "w_attn")) + _bseg(small, "b_attn")).reshape(SG, NH, NL * NP)
    attn = jax.nn.softmax(logits, axis=-1).reshape(SG, NH, NL, NP)

    acc = jnp.zeros((SG, NH, DH), _F32)
    for l, (H_, W_) in enumerate(SHAPES):
        x = r[:, l, 0][:, None, None] * W_ - 0.5 + off[:, :, l, :, 0]
        y = r[:, l, 1][:, None, None] * H_ - 0.5 + off[:, :, l, :, 1]
        x0 = jnp.floor(x)
        y0 = jnp.floor(y)
        a = attn[:, :, l, :]
        idxs, wgts = [], []
        for dx, dy in ((0, 0), (1, 0), (0, 1), (1, 1)):
            xi = x0 + dx
            yi = y0 + dy
            w = (1.0 - jnp.abs(x - xi)) * (1.0 - jnp.abs(y - yi)) * a
            valid = (xi >= 0) & (xi < W_) & (yi >= 0) & (yi < H_)
            wgts.append(jnp.where(valid, w, 0.0))
            idxs.append((_LVL_START[l] + jnp.clip(yi, 0, H_ - 1) * W_
                         + jnp.clip(xi, 0, W_ - 1)).astype(jnp.int32))
        idx = jnp.stack(idxs, 2)                      # [SG, NH, 4, NP]
        wgt = jnp.stack(wgts, 2)
        idx_t = idx.transpose(0, 2, 3, 1).reshape(SG * 4 * NP, NH, 1)
        wgt_t = wgt.transpose(0, 2, 3, 1).reshape(SG * 4 * NP, NH, 1)
        g = jnp.take_along_axis(vg, idx_t, axis=0, mode="promise_in_bounds")
        acc = acc + (g.astype(_F32) * wgt_t).reshape(SG, 4 * NP, NH, DH).sum(1)

    ca = mm(acc.reshape(SG, D).astype(_BF), _wseg(wall, "w_out")) + _bseg(small, "b_out")
    x = s_ + ca
    m = x.mean(-1, keepdims=True)
    v = ((x - m) ** 2).mean(-1, keepdims=True)
    x1 = (x - m) * jax.lax.rsqrt(v + 1e-5) * _bseg(small, "ln1_w") + _bseg(small, "ln1_b")
    h = jax.nn.relu(mm(x1.astype(_BF), _wseg(wall, "w_ff1")) + _bseg(small, "b_ff1"))
    ff = mm(h.astype(_BF), _wseg(wall, "w_ff2")) + _bseg(small, "b_ff2")
    y2 = x1 + ff
    m2 = y2.mean(-1, keepdims=True)
    v2 = ((y2 - m2) ** 2).mean(-1, keepdims=True)
    z = (y2 - m2) * jax.lax.rsqrt(v2 + 1e-5)          # pre-affine LN2 output
    return jnp.rint(jnp.clip(z, -Z_CLIP, Z_CLIP) * Z_SCALE).astype(jnp.int8)


@functools.lru_cache(maxsize=1)
def _stage0_prog():
    """Value projection + all-gather + group-0 compute in one program."""
    mesh = _mesh()

    def body(*args):
        # args: G src groups [1,1,SG,D] bf16, wall [1,_WTOT] bf16,
        # small [_BTOT] f32, aux0 [1,1,SG,_AUX_B] uint8
        srcs, wall, small, aux = args[:G], args[G][0], args[G + 1], args[G + 2]
        src_c = jnp.concatenate([s[0, 0] for s in srcs], 0).astype(_BF)
        v = jnp.dot(src_c, _wseg(wall, "w_value"), preferred_element_type=_F32)
        v = (v * _bseg(small, "s_src")[0] + _bseg(small, "b_value")).astype(_BF)
        vg = jax.lax.all_gather(v, "c", axis=0, tiled=True)   # [SPAD, D]
        z = _group_body(vg.reshape(SPAD, NH, DH), wall, small,
                        srcs[0][0, 0], aux[0, 0])
        return vg[None], z[None, None]

    fn = shard_map(
        body, mesh=mesh,
        in_specs=(P("b", "c"),) * G + (P(None), P(None), P("b", "c")),
        out_specs=(P("b"), P("b", "c")),
        check_rep=False,
    )
    return jax.jit(fn)


@functools.lru_cache(maxsize=1)
def _stage_prog():
    mesh = _mesh()

    def body(vg, wall, src, aux, small):
        z = _group_body(vg[0].reshape(SPAD, NH, DH), wall[0], small,
                        src[0, 0], aux[0, 0])
        return z[None, None]

    fn = shard_map(
        body, mesh=mesh,
        in_specs=(P("b"), P(None), P("b", "c"), P("b", "c"), P(None)),
        out_specs=P("b", "c"),
        check_rep=False,
    )
    return jax.jit(fn)


def _pad_tok(a):
    pad = SPAD - S
    if pad == 0:
        return a
    return np.concatenate([a, np.zeros((B, pad) + a.shape[2:], a.dtype)], 1)


def kernel(**inputs):
    f32 = lambda k: np.asarray(inputs[k], np.float32)
    mesh = _mesh()
    sh = NamedSharding(mesh, P("b", "c"))
    sh_rep = NamedSharding(mesh, P(None))

    # 1. weights first (small; gather program overlaps the src upload)
    wmat = np.concatenate([
        np.asarray(inputs[name], np.float32).ravel() for name in
        ("w_value", "w_off", "w_attn", "w_out", "w_ff1", "w_ff2")
    ]).astype(_BF).reshape(B, NCHUNK, _WTOT // 8)
    wmat_d = jax.device_put(wmat, sh)
    wall_d = _wgather_prog()(wmat_d)

    # 2. src groups on the wire as early as possible; core c group g owns
    # batch rows [c*SC + g*SG, c*SC + (g+1)*SG) so the stage-0 concat over
    # groups rebuilds each core's contiguous slice and the all-gather
    # rebuilds raster order
    src_f = f32("src")
    s_src = float(np.abs(src_f).max()) / 127.0
    if s_src == 0.0:
        s_src = 1.0
    src_q = np.clip(np.rint(src_f * (1.0 / s_src)), -127, 127).astype(np.int8)
    src_h = _pad_tok(src_q).reshape(B, NCHUNK, G, SG, D)
    src_gd = [jax.device_put(np.ascontiguousarray(src_h[:, :, g]), sh)
              for g in range(G)]

    pos = f32("pos")
    s_pos = float(np.abs(pos).max()) / 7.0
    if s_pos == 0.0:
        s_pos = 1.0
    small = np.concatenate([
        np.asarray(inputs[name], np.float32).ravel() for name in
        ("b_value", "b_off", "b_attn", "b_out", "b_ff1", "b_ff2",
         "ln1_w", "ln1_b")
    ] + [np.array([s_pos], np.float32), np.array([s_src], np.float32)])
    small_d = jax.device_put(small, sh_rep)

    # 3. per-group aux = [pos int4 | ref hi | ref lo] uint8
    q4 = (np.clip(np.rint(pos * (1.0 / s_pos)), -7, 7) + 8.0).astype(np.uint8)
    packed = (q4[..., 0::2] << 4) | q4[..., 1::2]          # [B, S, D//2]
    ref_u = np.rint(np.clip(f32("reference_points"), 0.0, 1.0) * 65535.0)
    ref_u = ref_u.astype(np.uint16).reshape(B, S, _REF_B)
    aux = np.concatenate(
        [packed, (ref_u >> 8).astype(np.uint8), (ref_u & 255).astype(np.uint8)],
        axis=2)                                             # [B, S, _AUX_B]
    aux = _pad_tok(aux).reshape(B, NCHUNK, G, SG, _AUX_B)

    aux0_d = jax.device_put(np.ascontiguousarray(aux[:, :, 0]), sh)

    vg_d, z0_d = _stage0_prog()(*src_gd, wall_d, small_d, aux0_d)
    z0_d.copy_to_host_async()
    outs = [z0_d]

    stage = _stage_prog()
    for g in range(1, G):
        aux_d = jax.device_put(np.ascontiguousarray(aux[:, :, g]), sh)
        o = stage(vg_d, wall_d, src_gd[g], aux_d, small_d)
        o.copy_to_host_async()
        outs.append(o)

    # 4. fetch int8 z, apply the LN2 affine on host in f32
    z = np.stack([np.asarray(o) for o in outs], 2)          # [B,NCHUNK,G,SG,D] int8
    z = z.astype(np.float32) * (Z_CLIP / 127.0)
    out = z.reshape(B, SPAD, D)[:, :S]
    out = out * f32("ln2_w") + f32("ln2_b")
    return out


# revision 12
# speedup vs baseline: 1.0049x; 1.0049x over previous
"""Deformable-DETR encoder layer, fused + pipelined on 8 trn2 NeuronCores.

All compute runs on-device (projections, softmax, bilinear gather-blend,
output projection, LayerNorms, FFN). The axon tunnel is half-duplex at
~55 MB/s, so total wire bytes dominate; the kernel minimizes them:

  up:   src bf16 (13.6MB) | per-group [pos int4-packed | ref uint16] as
        one uint8 buffer (3.9MB) | weights bf16 sharded (1.5MB,
        all-gathered on device) | biases f32
  down: pre-affine LN2 output as int8 (6.8MB, fixed 3.9-sigma scale);
        the final `z*ln2_w + ln2_b` affine is applied on host in f32.

Pipeline: a tiny program all-gathers the sharded weight upload while src
streams; stage 0 computes value = src@Wv, all-gathers it within each
batch group, and processes the first token group; stages 1..G-1 reuse
the gathered value, overlapping execution with the pos uploads and the
int8 output downloads.

Sharding: mesh (b=2, c=4) — data-parallel over batch, token-parallel
within a batch group; the flattened multi-scale value memory is
replicated within the group by the on-device all-gather.
"""
import functools

import jax
import jax.numpy as jnp
import numpy as np
from jax.experimental.shard_map import shard_map
from jax.sharding import Mesh, NamedSharding, PartitionSpec as P

SHAPES = ((100, 100), (50, 50), (25, 25), (13, 13))
B, D, NH, NL, NP, DFF = 2, 256, 8, 4, 4, 1024
DH = D // NH
S = sum(h * w for h, w in SHAPES)  # 13294
NCHUNK = 4
G = 4                              # pipeline groups per core
SC = ((S + NCHUNK * G - 1) // (NCHUNK * G)) * G  # 3324 per-core tokens
SPAD = SC * NCHUNK                 # 13296 padded per-batch tokens
SG = SC // G                       # 831 tokens per core per group

Z_CLIP = 3.9                       # int8 clip for the unit-variance LN output
Z_SCALE = 127.0 / Z_CLIP

# per-group aux buffer: [pos int4 packed D//2 | ref hi | ref lo] bytes per token
_POS_B = D // 2
_REF_B = NL * 2
_AUX_B = _POS_B + 2 * _REF_B

_LVL_START = np.cumsum([0] + [h * w for h, w in SHAPES])

_WSEG = {}
_off = 0
for _name, _shape in (("w_value", (D, D)), ("w_off", (D, D)), ("w_attn", (D, NH * NL * NP)),
                      ("w_out", (D, D)), ("w_ff1", (D, DFF)), ("w_ff2", (DFF, D))):
    _WSEG[_name] = (_off, _shape)
    _off += _shape[0] * _shape[1]
_WTOT = _off
assert _WTOT % 8 == 0

_BSEG = {}
_off = 0
for _name, _n in (("b_value", D), ("b_off", D), ("b_attn", NH * NL * NP), ("b_out", D),
                  ("b_ff1", DFF), ("b_ff2", D), ("ln1_w", D), ("ln1_b", D),
                  ("s_pos", 1), ("s_src", 1)):
    _BSEG[_name] = (_off, _n)
    _off += _n
_BTOT = _off

_BF = jnp.bfloat16
_F32 = jnp.float32


@functools.lru_cache(maxsize=1)
def _mesh():
    devs = np.array(jax.devices()[:8]).reshape(2, 4)
    return Mesh(devs, ("b", "c"))


def _wseg(wall, name):
    o, shp = _WSEG[name]
    return wall[o:o + shp[0] * shp[1]].reshape(shp)


def _bseg(small, name):
    o, n = _BSEG[name]
    return small[o:o + n]


@functools.lru_cache(maxsize=1)
def _wgather_prog():
    mesh = _mesh()

    def body(wmat):
        wall_c = jax.lax.all_gather(wmat[0, 0], "c", axis=0)
        wall = jax.lax.all_gather(wall_c, "b", axis=0).reshape(-1)
        return wall[None]

    fn = shard_map(body, mesh=mesh, in_specs=(P("b", "c"),),
                   out_specs=P(None), check_rep=False)
    return jax.jit(fn)


def _group_body(vg, wall, small, s_q, aux):
    """Per-group compute: q/off/attn, bilinear gather-blend, out-proj,
    LN1, FFN, pre-affine LN2 -> int8. All inputs are per-core blocks."""
    s_pos = _bseg(small, "s_pos")[0]
    s_src = _bseg(small, "s_src")[0]
    s_ = s_q.astype(_F32) * s_src
    u = aux[:, :_POS_B]
    hi = (jnp.right_shift(u, 4)).astype(_F32) - 8.0
    lo = (jnp.bitwise_and(u, 15)).astype(_F32) - 8.0
    pq = jnp.stack([hi, lo], -1).reshape(SG, D) * s_pos
    rhi = aux[:, _POS_B:_POS_B + _REF_B].astype(_F32)
    rlo = aux[:, _POS_B + _REF_B:].astype(_F32)
    r = ((rhi * 256.0 + rlo) * (1.0 / 65535.0)).reshape(SG, NL, 2)

    q = (s_ + pq).astype(_BF)
    mm = lambda a, w: jnp.dot(a, w, preferred_element_type=_F32)
    off = (mm(q, _wseg(wall, "w_off")) + _bseg(small, "b_off")).reshape(SG, NH, NL, NP, 2)
    logits = (mm(q, _wseg(wall, "w_attn")) + _bseg(small, "b_attn")).reshape(SG, NH, NL * NP)
    attn = jax.nn.softmax(logits, axis=-1).reshape(SG, NH, NL, NP)

    acc = jnp.zeros((SG, NH, DH), _F32)
    for l, (H_, W_) in enumerate(SHAPES):
        x = r[:, l, 0][:, None, None] * W_ - 0.5 + off[:, :, l, :, 0]
        y = r[:, l, 1][:, None, None] * H_ - 0.5 + off[:, :, l, :, 1]
        x0 = jnp.floor(x)
        y0 = jnp.floor(y)
        a = attn[:, :, l, :]
        idxs, wgts = [], []
        for dx, dy in ((0, 0), (1, 0), (0, 1), (1, 1)):
            xi = x0 + dx
            yi = y0 + dy
            w = (1.0 - jnp.abs(x - xi)) * (1.0 - jnp.abs(y - yi)) * a
            valid = (xi >= 0) & (xi < W_) & (yi >= 0) & (yi < H_)
            wgts.append(jnp.where(valid, w, 0.0))
            idxs.append((_LVL_START[l] + jnp.clip(yi, 0, H_ - 1) * W_
                         + jnp.clip(xi, 0, W_ - 1)).astype(jnp.int32))
        idx = jnp.stack(idxs, 2)                      # [SG, NH, 4, NP]
        wgt = jnp.stack(wgts, 2)
        idx_t = idx.transpose(0, 2, 3, 1).reshape(SG * 4 * NP, NH, 1)
        wgt_t = wgt.transpose(0, 2, 3, 1).reshape(SG * 4 * NP, NH, 1)
        g = jnp.take_along_axis(vg, idx_t, axis=0, mode="promise_in_bounds")
        acc = acc + (g.astype(_F32) * wgt_t).reshape(SG, 4 * NP, NH, DH).sum(1)

    ca = mm(acc.reshape(SG, D).astype(_BF), _wseg(wall, "w_out")) + _bseg(small, "b_out")
    x = s_ + ca
    m = x.mean(-1, keepdims=True)
    v = ((x - m) ** 2).mean(-1, keepdims=True)
    x1 = (x - m) * jax.lax.rsqrt(v + 1e-5) * _bseg(small, "ln1_w") + _bseg(small, "ln1_b")
    h = jax.nn.relu(mm(x1.astype(_BF), _wseg(wall, "w_ff1")) + _bseg(small, "b_ff1"))
    ff = mm(h.astype(_BF), _wseg(wall, "w_ff2")) + _bseg(small, "b_ff2")
    y2 = x1 + ff
    m2 = y2.mean(-1, keepdims=True)
    v2 = ((y2 - m2) ** 2).mean(-1, keepdims=True)
    z = (y2 - m2) * jax.lax.rsqrt(v2 + 1e-5)          # pre-affine LN2 output
    return jnp.rint(jnp.clip(z, -Z_CLIP, Z_CLIP) * Z_SCALE).astype(jnp.int8)


@functools.lru_cache(maxsize=1)
def _stage0_prog():
    """Value projection + all-gather + group-0 compute in one program."""
    mesh = _mesh()

    def body(*args):
        # args: G src groups [1,1,SG,D] bf16, wall [1,_WTOT] bf16,
        # small [_BTOT] f32, aux0 [1,1,SG,_AUX_B] uint8
        srcs, wall, small, aux = args[:G], args[G][0], args[G + 1], args[G + 2]
        src_c = jnp.concatenate([s[0, 0] for s in srcs], 0).astype(_BF)
        v = jnp.dot(src_c, _wseg(wall, "w_value"), preferred_element_type=_F32)
        v = (v * _bseg(small, "s_src")[0] + _bseg(small, "b_value")).astype(_BF)
        vg = jax.lax.all_gather(v, "c", axis=0, tiled=True)   # [SPAD, D]
        z = _group_body(vg.reshape(SPAD, NH, DH), wall, small,
                        srcs[0][0, 0], aux[0, 0])
        return vg[None], z[None, None]

    fn = shard_map(
        body, mesh=mesh,
        in_specs=(P("b", "c"),) * G + (P(None), P(None), P("b", "c")),
        out_specs=(P("b"), P("b", "c")),
        check_rep=False,
    )
    return jax.jit(fn)


@functools.lru_cache(maxsize=1)
def _stage_prog():
    mesh = _mesh()

    def body(vg, wall, src, aux, small):
        z = _group_body(vg[0].reshape(SPAD, NH, DH), wall[0], small,
                        src[0, 0], aux[0, 0])
        return z[None, None]

    fn = shard_map(
        body, mesh=mesh,
        in_specs=(P("b"), P(None), P("b", "c"), P("b", "c"), P(None)),
        out_specs=P("b", "c"),
        check_rep=False,
    )
    return jax.jit(fn)


def _pad_tok(a):
    pad = SPAD - S
    if pad == 0:
        return a
    return np.concatenate([a, np.zeros((B, pad) + a.shape[2:], a.dtype)], 1)


def kernel(**inputs):
    f32 = lambda k: np.asarray(inputs[k], np.float32)
    mesh = _mesh()
    sh = NamedSharding(mesh, P("b", "c"))
    sh_rep = NamedSharding(mesh, P(None))

    # 1. weights first (small; gather program overlaps the src upload)
    wmat = np.concatenate([
        np.asarray(inputs[name], np.float32).ravel() for name in
        ("w_value", "w_off", "w_attn", "w_out", "w_ff1", "w_ff2")
    ]).astype(_BF).reshape(B, NCHUNK, _WTOT // 8)
    wmat_d = jax.device_put(wmat, sh)
    wall_d = _wgather_prog()(wmat_d)

    # 2. src groups on the wire as early as possible; core c group g owns
    # batch rows [c*SC + g*SG, c*SC + (g+1)*SG) so the stage-0 concat over
    # groups rebuilds each core's contiguous slice and the all-gather
    # rebuilds raster order
    src_f = f32("src")
    s_src = float(np.abs(src_f).max()) / 127.0
    if s_src == 0.0:
        s_src = 1.0
    # quantize per group and enqueue each put immediately so the host-side
    # int8 conversion overlaps the wire instead of preceding it
    src_p = _pad_tok(src_f).reshape(B, NCHUNK, G, SG, D)
    inv_s = 1.0 / s_src
    src_gd = []
    for g in range(G):
        q = np.clip(np.rint(src_p[:, :, g] * inv_s), -127, 127).astype(np.int8)
        src_gd.append(jax.device_put(q, sh))

    pos = f32("pos")
    s_pos = float(np.abs(pos).max()) / 7.0
    if s_pos == 0.0:
        s_pos = 1.0
    small = np.concatenate([
        np.asarray(inputs[name], np.float32).ravel() for name in
        ("b_value", "b_off", "b_attn", "b_out", "b_ff1", "b_ff2",
         "ln1_w", "ln1_b")
    ] + [np.array([s_pos], np.float32), np.array([s_src], np.float32)])
    small_d = jax.device_put(small, sh_rep)

    # 3. per-group aux = [pos int4 | ref hi | ref lo] uint8
    q4 = (np.clip(np.rint(pos * (1.0 / s_pos)), -7, 7) + 8.0).astype(np.uint8)
    packed = (q4[..., 0::2] << 4) | q4[..., 1::2]          # [B, S, D//2]
    ref_u = np.rint(np.clip(f32("reference_points"), 0.0, 1.0) * 65535.0)
    ref_u = ref_u.astype(np.uint16).reshape(B, S, _REF_B)
    aux = np.concatenate(
        [packed, (ref_u >> 8).astype(np.uint8), (ref_u & 255).astype(np.uint8)],
        axis=2)                                             # [B, S, _AUX_B]
    aux = _pad_tok(aux).reshape(B, NCHUNK, G, SG, _AUX_B)

    aux0_d = jax.device_put(np.ascontiguousarray(aux[:, :, 0]), sh)

    vg_d, z0_d = _stage0_prog()(*src_gd, wall_d, small_d, aux0_d)
    z0_d.copy_to_host_async()
    outs = [z0_d]

    stage = _stage_prog()
    for g in range(1, G):
        aux_d = jax.device_put(np.ascontiguousarray(aux[:, :, g]), sh)
        o = stage(vg_d, wall_d, src_gd[g], aux_d, small_d)
        o.copy_to_host_async()
        outs.append(o)

    # 4. fetch int8 z, apply the LN2 affine on host in f32
    z = np.stack([np.asarray(o) for o in outs], 2)          # [B,NCHUNK,G,SG,D] int8
    z = z.astype(np.float32) * (Z_CLIP / 127.0)
    out = z.reshape(B, SPAD, D)[:, :S]
    out = out * f32("ln2_w") + f32("ln2_b")
    return out
